# revision 17
# baseline (speedup 1.0000x reference)
"""CMPGNN message-passing kernel for 8 Trainium2 NeuronCores.

Strategy (node partitioning, per sharding hint):
- Nodes are globally sorted by in-degree and dealt into 98 "stripes" of
  1024; each stripe contributes 128 nodes to the same block index on every
  core.  This makes the per-block tile count T_s identical across cores
  (pure SPMD, one NEFF) and nearly eliminates padding (degrees within a
  stripe are near-equal).
- Edges are assigned to the core owning their target (col) node.  Within a
  target block, the t-th in-edge of col p sits at slot (tile t, partition
  p), so the scatter is a PSUM-accumulated matmul with a constant identity
  lhsT (no one-hot build needed) and h4[col] is a single per-block
  sequential load.
- Per-edge h3/hsum rows are fetched with per-tile indirect DMA row gathers
  from an all-gathered bf16 table [h3 | h3+h4] ([N,128]); h4 lives in a
  core-local table (never exchanged).  PSUM accumulation and norms stay f32
  (measured rel err ~1e-4 end to end).
- Per layer: build own table shard (matmul), AllGather shard -> full table,
  per-block edge pass (gather, dot+sigmoid gate on DVE/ACT, message build,
  identity-scatter matmul), add relu(Q@Wf.T), L2-normalize.  Layer 2 also
  emits unnormalized logits inside the edge pass (row scaling commutes
  through the logits matmul); scale+bias+log_softmax run as batched ops.
"""
import numpy as np

import concourse.bass as bass
import concourse.bacc as bacc
import concourse.tile as tile
from concourse import mybir
from concourse.bass_utils import run_bass_kernel_spmd

N, E, F_IN, H, C = 100000, 1250000, 512, 64, 40
NCORE = 8
P = 128
S = 98                 # blocks (stripes) per core
SHARD = S * P          # 12544 table rows per core (incl. pads)
NTAB = NCORE * SHARD   # 100352 global table rows
KCH = F_IN // P        # 4 k-chunks for the input matmul
GMAX = 16              # tiles per DVE batch group
F32 = mybir.dt.float32
AX = mybir.AxisListType
ALU = mybir.AluOpType
ACTF = mybir.ActivationFunctionType


def _prep(x, edge_index):
    row = np.asarray(edge_index[0], dtype=np.int64)
    col = np.asarray(edge_index[1], dtype=np.int64)
    deg = np.bincount(col, minlength=N)

    order = np.argsort(-deg, kind="stable")
    j = np.arange(N)
    s_pos = j // (NCORE * P)
    r = j % (NCORE * P)
    c_pos = r % NCORE
    p_pos = r // NCORE
    core_of = np.empty(N, np.int64)
    s_of = np.empty(N, np.int64)
    p_of = np.empty(N, np.int64)
    core_of[order] = c_pos
    s_of[order] = s_pos
    p_of[order] = p_pos
    srow = s_of * P + p_of                 # row within core shard
    # chunk-major pid: table rows ordered [chunk, core, srow%HS] so each
    # half-shard AllGather output is contiguous (lets cc overlap table build)
    HS = SHARD // 2
    chunk = srow // HS
    pid = chunk * (NCORE * HS) + core_of * HS + (srow % HS)

    T = np.zeros(S, np.int64)
    np.maximum.at(T, s_of, deg)
    T = np.maximum(T, 1)
    O = np.concatenate([[0], np.cumsum(T)[:-1]])
    tiles_total = int(T.sum())

    # slot assignment: t-th in-edge of col -> tile O[s]+t, partition p(col)
    e_order = np.argsort(col, kind="stable")
    col_s = col[e_order]
    row_s = row[e_order]
    starts = np.searchsorted(col_s, np.arange(N))
    rank = np.arange(E) - starts[col_s]
    ec = core_of[col_s]
    tcol = O[s_of[col_s]] + rank
    epart = p_of[col_s]
    gid = pid[row_s]

    idx_hosts = []
    for c in range(NCORE):
        # pad -> this core's zero row (srow SHARD-1), in chunk-major pid space
        pad_pid = NCORE * HS + c * HS + (HS - 1)
        a = np.full((P, tiles_total), pad_pid, np.int32)
        m = ec == c
        a[epart[m], tcol[m]] = gid[m]
        idx_hosts.append(a)

    node_of = np.full((NCORE, SHARD), -1, np.int64)
    node_of[core_of, srow] = np.arange(N)

    xT_hosts = []
    for c in range(NCORE):
        ids = node_of[c]
        xs = np.zeros((SHARD, F_IN), np.float32)
        mreal = ids >= 0
        xs[mreal] = x[ids[mreal]]
        xT_hosts.append(np.ascontiguousarray(xs.T))
    return idx_hosts, xT_hosts, node_of, T, O, tiles_total


def _build(T, O, tiles_total, do_gather=True, local_tab=False, bf16=False, gather_every=1, nq=1, gmax=GMAX, deep=False):
    nc = bacc.Bacc("TRN2", target_bir_lowering=False, num_swdge_queues=nq)
    TDT = mybir.dt.bfloat16 if bf16 else F32

    xT_e = nc.declare_dram_parameter("xT", [F_IN, SHARD], TDT, isOutput=False)
    idx_e = nc.declare_dram_parameter("idx", [P, tiles_total], mybir.dt.int32, isOutput=False)
    winT_e = nc.declare_dram_parameter("winT", [F_IN, H], F32, isOutput=False)
    bin_e = nc.declare_dram_parameter("b_in", [1, H], F32, isOutput=False)
    bout_e = nc.declare_dram_parameter("b_out", [1, C], F32, isOutput=False)
    woutT_e = nc.declare_dram_parameter("woutT", [H, C], F32, isOutput=False)
    wp_e = [nc.declare_dram_parameter(f"wp{l}", [H, 3 * H], F32, isOutput=False) for l in range(2)]
    wfT_e = [nc.declare_dram_parameter(f"wfT{l}", [H, H], F32, isOutput=False) for l in range(2)]
    out_e = nc.declare_dram_parameter("out", [SHARD, C], F32, isOutput=True)

    tabA_own = [nc.dram_tensor(f"tabA_own{l}", [SHARD, 2 * H], TDT) for l in range(2)]
    tabB_own = [nc.dram_tensor(f"tabB_own{l}", [SHARD, H], TDT) for l in range(2)]
    tabA_full = [nc.dram_tensor(f"tabA_full{l}", [NTAB, 2 * H], TDT, addr_space="Shared")
                 for l in range(2)]

    with tile.TileContext(nc) as tc:
        with (
            tc.tile_pool(name="const", bufs=1) as cp,
            tc.tile_pool(name="big", bufs=1) as bp,
            tc.tile_pool(name="stats", bufs=10) as stp,
            tc.tile_pool(name="xt", bufs=4 if deep else 3) as xp,
            tc.tile_pool(name="tb", bufs=6 if deep else 4) as tbp,
            tc.tile_pool(name="gat", bufs=6 if deep else 4) as gp,
            tc.tile_pool(name="mid", bufs=4 if deep else 3) as mp,
            tc.tile_pool(name="small", bufs=8 if deep else 4) as sp,
            tc.tile_pool(name="ps", bufs=4, space="PSUM") as ps,
            tc.tile_pool(name="psagg", bufs=4, space="PSUM") as psa,
        ):
            # ---- constants ----
            idxbuf = cp.tile([P, tiles_total], mybir.dt.int32)
            nc.sync.dma_start(out=idxbuf[:, :], in_=idx_e.ap())
            ident = cp.tile([P, P], F32)
            nc.gpsimd.memset(ident[:, :], 0.0)
            nc.gpsimd.affine_select(out=ident[:, :], in_=ident[:, :],
                                    compare_op=ALU.not_equal, fill=1.0, base=0,
                                    pattern=[[-1, P]], channel_multiplier=1)
            if bf16:
                ident_mm = cp.tile([P, P], TDT)
                nc.vector.tensor_copy(out=ident_mm[:, :], in_=ident[:, :])
            else:
                ident_mm = ident
            ones_row = cp.tile([1, P], F32)
            nc.vector.memset(ones_row[:, :], 1.0)
            winT_f = cp.tile([P, KCH, H], F32)
            nc.sync.dma_start(out=winT_f[:, :, :], in_=winT_e.ap().rearrange("(k p) h -> p k h", p=P))
            if bf16:
                winT = cp.tile([P, KCH, H], TDT)
                nc.vector.tensor_copy(out=winT[:, :, :], in_=winT_f[:, :, :])
            else:
                winT = winT_f
            b_in = cp.tile([1, H], F32)
            nc.sync.dma_start(out=b_in[:, :], in_=bin_e.ap())
            b_out = cp.tile([1, C], F32)
            nc.sync.dma_start(out=b_out[:, :], in_=bout_e.ap())
            woutT = cp.tile([H, C], F32)
            nc.sync.dma_start(out=woutT[:, :], in_=woutT_e.ap())
            wp = []
            wfT = []
            for l in range(2):
                w1 = cp.tile([H, 3 * H], F32, tag=f"wp{l}")
                nc.sync.dma_start(out=w1[:, :], in_=wp_e[l].ap())
                wp.append(w1)
                w2 = cp.tile([H, H], F32, tag=f"wfT{l}")
                nc.sync.dma_start(out=w2[:, :], in_=wfT_e[l].ap())
                wfT.append(w2)

            bo_ps = ps.tile([P, C], F32, tag="mm")
            nc.tensor.matmul(out=bo_ps[:, :], lhsT=ones_row[:, :], rhs=b_out[:, :],
                             start=True, stop=True)
            b_out_full = cp.tile([P, C], F32)
            nc.vector.tensor_copy(out=b_out_full[:, :], in_=bo_ps[:, :])

            # ---- persistent buffers ----
            QT = bp.tile([H, S, P], F32)       # Q^T per block (lhsT layout)
            Qb = bp.tile([P, S, H], F32)       # Q per block (node-major)
            LG = bp.tile([P, S, C], F32)       # logits

            # ---- phase 1: Q^T = W_in @ x^T (+ b_in) ----
            for s in range(S):
                qt_ps = ps.tile([H, P], F32, tag="mm")
                xt = xp.tile([P, KCH, P], TDT)
                nc.sync.dma_start(
                    out=xt[:, :, :],
                    in_=xT_e.ap().rearrange("(k p) n -> p k n", p=P)[:, :, s * P:(s + 1) * P])
                for kc in range(KCH):
                    nc.tensor.matmul(out=qt_ps[:, :], lhsT=winT[:, kc, :], rhs=xt[:, kc, :],
                                     start=(kc == 0), stop=False)
                nc.tensor.matmul(out=qt_ps[:, :], lhsT=b_in[:, :], rhs=ones_row[:, :],
                                 start=False, stop=True)
                if s % 2 == 0:
                    nc.vector.tensor_copy(out=QT[:, s, :], in_=qt_ps[:, :])
                else:
                    nc.scalar.activation(out=QT[:, s, :], in_=qt_ps[:, :], func=ACTF.Copy)

            for l in range(2):
                # zero the pad region so pad-slot gathers stay zero
                nc.vector.memset(QT[:, S - 1, (672 // NCORE):], 0.0)

                # ---- table build + allgather ----
                for s in range(S):
                    tb_ps = ps.tile([P, 3 * H], F32, tag="mm")
                    nc.tensor.matmul(out=tb_ps[:, :], lhsT=QT[:, s, :], rhs=wp[l][:, :],
                                     start=True, stop=True)
                    tba = tbp.tile([P, 2 * H], TDT, tag="tba")
                    tbb = tbp.tile([P, H], TDT, tag="tbb")
                    if s % 2 == 0:
                        nc.vector.tensor_copy(out=tba[:, :], in_=tb_ps[:, 0:2 * H])
                        nc.scalar.activation(out=tbb[:, :], in_=tb_ps[:, 2 * H:3 * H], func=ACTF.Copy)
                    else:
                        nc.scalar.activation(out=tba[:, :], in_=tb_ps[:, 0:2 * H], func=ACTF.Copy)
                        nc.vector.tensor_copy(out=tbb[:, :], in_=tb_ps[:, 2 * H:3 * H])
                    nc.sync.dma_start(out=tabA_own[l].ap()[s * P:(s + 1) * P, :], in_=tba[:, :])
                    nc.sync.dma_start(out=tabB_own[l].ap()[s * P:(s + 1) * P, :], in_=tbb[:, :])

                if not local_tab:
                    HS = SHARD // 2
                    for j in range(2):
                        nc.gpsimd.collective_compute(
                            "AllGather", ALU.bypass,
                            replica_groups=[list(range(NCORE))],
                            ins=[tabA_own[l].ap()[j * HS:(j + 1) * HS, :]],
                            outs=[tabA_full[l].ap()[j * NCORE * HS:(j + 1) * NCORE * HS, :]],
                        )

                normsq = stp.tile([P, S], F32, tag="normsq")

                # ---- edge pass ----
                for s in range(S):
                    ts = int(T[s])
                    o0 = int(O[s])
                    h4b = sp.tile([P, H], TDT, tag="h4b")
                    nc.sync.dma_start(out=h4b[:, :], in_=tabB_own[l].ap()[s * P:(s + 1) * P, :])

                    agg = psa.tile([P, H], F32, tag="agg")
                    for g0 in range(0, ts, gmax):
                        k = min(gmax, ts - g0)
                        g = gp.tile([P, gmax, 2 * H], TDT, tag="g")
                        gtab = tabA_own[l] if local_tab else tabA_full[l]
                        if do_gather:
                            for t in range(0, k, gather_every):
                                gi = nc.gpsimd.indirect_dma_start(
                                    out=g[:, t, :], out_offset=None,
                                    in_=gtab.ap(),
                                    in_offset=bass.IndirectOffsetOnAxis(
                                        ap=idxbuf[:, o0 + g0 + t:o0 + g0 + t + 1], axis=0))
                                if nq > 1 and (t % nq):
                                    gi.ins.queue = f"qPoolDynamic{t % nq}"
                        prod = mp.tile([P, gmax, H], TDT, tag="prod")
                        nc.vector.tensor_tensor(
                            out=prod[:, :k, :], in0=g[:, :k, 0:H],
                            in1=h4b[:, None, :].to_broadcast([P, k, H]), op=ALU.mult)
                        d = sp.tile([P, gmax], F32, tag="d")
                        nc.vector.tensor_reduce(out=d[:, :k], in_=prod[:, :k, :],
                                                axis=AX.X, op=ALU.add)
                        sg = sp.tile([P, gmax], TDT, tag="sg")
                        nc.scalar.activation(out=sg[:, :k], in_=d[:, :k], func=ACTF.Sigmoid)
                        # t = sigmoid(d) * hsum  (reuse prod)
                        nc.vector.tensor_tensor(
                            out=prod[:, :k, :], in0=g[:, :k, H:2 * H],
                            in1=sg[:, :k].to_broadcast([P, k, H]), op=ALU.mult)
                        msg = mp.tile([P, gmax, H], TDT, tag="msg")
                        nc.vector.tensor_tensor(out=msg[:, :k, :], in0=g[:, :k, 0:H],
                                                in1=prod[:, :k, :], op=ALU.subtract)
                        for t in range(k):
                            nc.tensor.matmul(out=agg[:, :], lhsT=ident_mm[:, :], rhs=msg[:, t, :],
                                             start=(g0 + t == 0), stop=(g0 + t == ts - 1))

                    hl_ps = ps.tile([P, H], F32, tag="mm")
                    nc.tensor.matmul(out=hl_ps[:, :], lhsT=QT[:, s, :], rhs=wfT[l][:, :],
                                     start=True, stop=True)
                    hl = sp.tile([P, H], F32, tag="hl")
                    nc.scalar.activation(out=hl[:, :], in_=hl_ps[:, :], func=ACTF.Relu)
                    nc.vector.tensor_add(out=Qb[:, s, :], in0=hl[:, :], in1=agg[:, :])
                    qtr = sp.tile([P, H], F32, tag="qtr")
                    nc.scalar.activation(out=qtr[:, :], in_=Qb[:, s, :], func=ACTF.Square,
                                         accum_out=normsq[:, s:s + 1])
                    if l == 1:
                        # unnormalized logits; row-scaling commutes through the
                        # matmul, so scale+bias happen batched after the norms
                        tr_ps = ps.tile([H, P], F32, tag="mm")
                        nc.tensor.transpose(out=tr_ps[:, :], in_=Qb[:, s, :], identity=ident[:, :])
                        q2t = sp.tile([H, P], F32, tag="q2t")
                        if s % 2 == 0:
                            nc.vector.tensor_copy(out=q2t[:, :], in_=tr_ps[:, :])
                        else:
                            nc.scalar.activation(out=q2t[:, :], in_=tr_ps[:, :], func=ACTF.Copy)
                        lg_ps = ps.tile([P, C], F32, tag="mm")
                        nc.tensor.matmul(out=lg_ps[:, :], lhsT=q2t[:, :], rhs=woutT[:, :],
                                         start=True, stop=True)
                        if s % 2 == 0:
                            nc.scalar.activation(out=LG[:, s, :], in_=lg_ps[:, :], func=ACTF.Copy)
                        else:
                            nc.vector.tensor_copy(out=LG[:, s, :], in_=lg_ps[:, :])

                # ---- normalize ----
                ns2 = stp.tile([P, S], F32, tag="ns2")
                nc.vector.tensor_scalar_max(out=ns2[:, :], in0=normsq[:, :], scalar1=1e-24)
                nrm = stp.tile([P, S], F32, tag="nrm")
                nc.scalar.activation(out=nrm[:, :], in_=ns2[:, :], func=ACTF.Sqrt)
                inv = stp.tile([P, S], F32, tag="inv")
                nc.vector.reciprocal(out=inv[:, :], in_=nrm[:, :])
                if l == 0:
                    nc.vector.tensor_tensor(out=Qb[:, :, :], in0=Qb[:, :, :],
                                            in1=inv[:, :].to_broadcast([P, S, H]), op=ALU.mult)
                else:
                    nc.vector.tensor_tensor(out=LG[:, :, :], in0=LG[:, :, :],
                                            in1=inv[:, :].to_broadcast([P, S, C]), op=ALU.mult)
                    nc.vector.tensor_tensor(
                        out=LG[:, :, :], in0=LG[:, :, :],
                        in1=b_out_full[:, None, :].to_broadcast([P, S, C]), op=ALU.add)

                if l == 0:
                    # rebuild Q^T for the next layer
                    for s in range(S):
                        tr_ps = ps.tile([H, P], F32, tag="mm")
                        nc.tensor.transpose(out=tr_ps[:, :], in_=Qb[:, s, :], identity=ident[:, :])
                        if s % 2 == 0:
                            nc.vector.tensor_copy(out=QT[:, s, :], in_=tr_ps[:, :])
                        else:
                            nc.scalar.activation(out=QT[:, s, :], in_=tr_ps[:, :], func=ACTF.Copy)

            # ---- log_softmax ----
            mx = stp.tile([P, S], F32, tag="mx")
            nc.vector.tensor_reduce(out=mx[:, :], in_=LG[:, :, :], axis=AX.X, op=ALU.max)
            nc.vector.tensor_tensor(out=LG[:, :, :], in0=LG[:, :, :],
                                    in1=mx[:, :].to_broadcast([P, S, C]), op=ALU.subtract)
            sume = stp.tile([P, S], F32, tag="sume")
            for s in range(S):
                etr = sp.tile([P, C], F32, tag="etr")
                nc.scalar.activation(out=etr[:, :], in_=LG[:, s, :], func=ACTF.Exp,
                                     accum_out=sume[:, s:s + 1])
            lse = stp.tile([P, S], F32, tag="lse")
            nc.scalar.activation(out=lse[:, :], in_=sume[:, :], func=ACTF.Ln)
            nc.vector.tensor_tensor(out=LG[:, :, :], in0=LG[:, :, :],
                                    in1=lse[:, :].to_broadcast([P, S, C]), op=ALU.subtract)
            nc.sync.dma_start(out=out_e.ap().rearrange("(s p) c -> p s c", p=P), in_=LG[:, :, :])

    nc.compile()
    return nc


_CACHE = {}


def _make_inmaps(inputs):
    """Host-side sharding: returns (nc, in_maps, node_of)."""
    x = np.asarray(inputs["x"], np.float32)
    edge_index = np.asarray(inputs["edge_index"])
    idx_hosts, xT_hosts, node_of, T, O, tiles_total = _prep(x, edge_index)

    key = ("nc", tiles_total, tuple(T.tolist()))
    if key not in _CACHE:
        # bf16 gather tables: halves gather/collective bytes and doubles DVE
        # throughput; PSUM accumulation stays f32 (measured rel err ~1e-4).
        _CACHE[key] = _build(T, O, tiles_total, bf16=True)
    nc = _CACHE[key]

    W1 = [np.asarray(inputs["W1_0"], np.float32), np.asarray(inputs["W1_1"], np.float32)]
    W2 = [np.asarray(inputs["W2_0"], np.float32), np.asarray(inputs["W2_1"], np.float32)]
    Wf = [np.asarray(inputs["Wf_0"], np.float32), np.asarray(inputs["Wf_1"], np.float32)]
    common = {
        "winT": np.ascontiguousarray(np.asarray(inputs["W_in"], np.float32).T),
        "b_in": np.asarray(inputs["b_in"], np.float32).reshape(1, H),
        "b_out": np.asarray(inputs["b_out"], np.float32).reshape(1, C),
        "woutT": np.ascontiguousarray(np.asarray(inputs["W_out"], np.float32).T),
    }
    for l in range(2):
        common[f"wp{l}"] = np.ascontiguousarray(
            np.concatenate([W1[l].T, (W1[l] + W2[l]).T, W2[l].T], axis=1))
        common[f"wfT{l}"] = np.ascontiguousarray(Wf[l].T)

    import ml_dtypes
    in_maps = [dict(common, xT=xT_hosts[c].astype(ml_dtypes.bfloat16), idx=idx_hosts[c])
               for c in range(NCORE)]
    return nc, in_maps, node_of


def kernel(**inputs):
    nc, in_maps, node_of = _make_inmaps(inputs)
    res = run_bass_kernel_spmd(nc, in_maps, core_ids=list(range(NCORE)))

    out = np.empty((N, C), np.float32)
    for c in range(NCORE):
        ids = node_of[c]
        m = ids >= 0
        out[ids[m]] = res.results[c]["out"][m]
    return out
